# revision 25
# baseline (speedup 1.0000x reference)
"""Trainium2 Bass kernel for 2-layer multi-head GAT (nn_GAT_38551626449703).

Strategy (8 NeuronCores, SPMD):
  - Nodes are partitioned uniformly: core k owns nodes [k*NPC, (k+1)*NPC).
  - Edges are sharded by OWNER OF src (softmax groups by src stay core-local).
  - Per core, edges are grouped into 128-node windows; each window has G groups
    of 128 edge-slots, split into an A-section (dst < 32767) and a B-section
    (dst >= 32767) so table rows fit int16 indices for dma_gather.
  - Gather tables (dma_gather rows must be 256B-multiples):
      TW   [N+2, 384] bf16: els 0:256 Wh (4 heads), els 256:264 = s2 as 4xf32,
                            rest pad. Rows n+1; rows 0 / N+1 are sentinels
                            with s2 = -1e30 (768B rows).
      TS1  [NPC, 64] f32:  els 0:4 = s1 (by src, core-local; 256B rows)
      T2M  [N+2, 128] bf16: els 0:64 Wh2, els 64:66 = s2o as 1xf32 (256B rows)
      T2S1 [NPC, 64] f32:  el 0 = s1o
  - Pad slots gather sentinel rows (s2 = -1e30 -> exp(e) == 0 exactly).
  - Per window: 2 dst-section gather calls + 1 src gather call,
    e = lrelu(s1+s2) (Act engine), ex = exp(e) -> bf16,
    R = [ex*Wh | ex] bf16 in place, one-hot(src) bf16 matmuls accumulate
    [u | denom] per node in PSUM across the window's G groups (1 cyc/row),
    then h' = u/denom, ELU (Act engine assisted).
  - Between layers only the compact T2M shard (1.6 MB bf16) is AllGathered.
  - Outputs (rows for owned nodes) are concatenated on the host.
"""

import os
import sys

import numpy as np

sys.path.insert(0, "/opt/trn_rl_repo")

import ml_dtypes  # noqa: E402

import concourse.bacc as bacc  # noqa: E402
import concourse.bass as bass  # noqa: E402
import concourse.tile as tile  # noqa: E402
from concourse import mybir  # noqa: E402
from concourse.masks import make_identity  # noqa: E402

F32 = mybir.dt.float32
BF16 = mybir.dt.bfloat16
I32 = mybir.dt.int32
I16 = mybir.dt.int16
AF = mybir.ActivationFunctionType
ALU = mybir.AluOpType
BF_NP = ml_dtypes.bfloat16

# Problem constants
N = 50000
E = 800000
F_IN = 128
HID = 64
HEADS = 4
OUT = 64
ALPHA = 0.2
CORES = 8

NEG = -1.0e30  # sentinel s2 -> exp(lrelu(s1+NEG)) == 0.0 in f32
HALF = 32767  # dst < HALF -> A section (table row dst+1 <= 32767)
NSWQ = int(os.environ.get("GAT_NSWQ", "4"))  # SWDGE queues
HOSTOH = bool(int(os.environ.get("GAT_HOSTOH", "1")))  # host-built one-hot
QG = int(os.environ.get("GAT_QG", "6"))  # groups per dma_gather call
UNIKA = bool(int(os.environ.get("GAT_UNIKA", "0")))  # uniform ka/kb
# SWDGE descriptor ring is dynamic_dma_scratch_size/16 = 1024 descs per
# queue; a single gather call must stay well under that (QG*128 <= 768).

# Tile assigns the 8 DMASW completion-sem lanes round-robin over Pool-engine
# DMAs regardless of SWDGE queue, but a lane must stay on ONE queue (ucode
# constraint; violations -> corrupted sync / device crash). With NSWQ > 1 we
# partition the lanes: queue q owns lanes [q*8//NSWQ, (q+1)*8//NSWQ).
if NSWQ > 1:
    import concourse.bass_isa as _bass_isa
    import concourse.tile_sem_assignment as _tsa

    _orig_assign_tick = _tsa.TileClockTick._assign_tick

    def _lane_partitioned_assign_tick(self, inst):
        if (
            isinstance(inst, _tsa.DMAInst)
            and inst.engine == mybir.EngineType.Pool
            and not isinstance(inst, _bass_isa.UserSyncedRemoteDMADescs)
        ):
            qn = getattr(inst, "queue_num", 0) or 0
            per = getattr(self, "_q_lane_ctr", None)
            if per is None:
                per = self._q_lane_ctr = {}
            lanes = 8 // NSWQ
            c = per.get(qn, 0)
            per[qn] = c + 1
            self.next_sw_dma_idx = qn * lanes + (c % lanes)
        return _orig_assign_tick(self, inst)

    if _tsa.TileClockTick._assign_tick is not _lane_partitioned_assign_tick:
        _tsa.TileClockTick._assign_tick = _lane_partitioned_assign_tick

RW = 384  # TW row elements (bf16) = 768B
R2W = 128  # T2M row elements (bf16) = 256B
TB = 4  # tiles per batched DMA (phases A/C)
WB = 3  # windows per gather batch (phases B/E)


class Cfg:
    def __init__(self, n, cores, kaw, kbw):
        assert n % cores == 0
        self.n = n
        self.cores = cores
        self.npc = n // cores
        self.nw = (self.npc + 127) // 128  # windows per core
        self.kaw = [int(v) for v in kaw]  # A-section groups, per window
        self.kbw = [int(v) for v in kbw]  # B-section groups, per window
        assert len(self.kaw) == self.nw and len(self.kbw) == self.nw
        self.g = max(a + b for a, b in zip(self.kaw, self.kbw))
        self.nb = (self.nw + WB - 1) // WB  # gather batches
        # per-batch layout: [A(w0)..A(wk) | B(w0)..B(wk)] group spans
        self.batches = []
        for w0, kw in _chunks(self.nw, WB):
            kas = self.kaw[w0 : w0 + kw]
            kbs = self.kbw[w0 : w0 + kw]
            sa = sum(kas)
            aoff = [sum(kas[:i]) for i in range(kw)]
            boff = [sa + sum(kbs[:i]) for i in range(kw)]
            self.batches.append(
                dict(w0=w0, kw=kw, kas=kas, kbs=kbs, aoff=aoff, boff=boff,
                     sa=sa, bg=sa + sum(kbs))
            )
        self.maxbg = max(b["bg"] for b in self.batches)
        self.key = (n, cores, tuple(self.kaw), tuple(self.kbw))


def _chunks(nt, step=TB):
    """[(t0, ntiles), ...] batches of `step` tiles."""
    out = []
    t = 0
    while t < nt:
        k = min(step, nt - t)
        out.append((t, k))
        t += k
    return out


def _gcalls(g0, g1):
    """Split groups [g0, g1) into dma_gather calls of <= QG groups."""
    out = []
    g = g0
    while g < g1:
        q = min(QG, g1 - g)
        out.append((g, q))
        g += q
    return out


def build_nc(cfg: Cfg, dbg: bool = False, reps=None, mock_d: bool = False):
    """Build the SPMD Bass program (one program, runs on all cores).

    mock_d: replace the AllGather with a local DMA (for single-core
    cost-model simulation only; wrong results on real multi-core runs)."""
    reps = reps or {}
    n, npc, NW = cfg.n, cfg.npc, cfg.nw
    MBG = cfg.maxbg
    ANYB = max(cfg.kbw) > 0
    NT1 = (n + 127) // 128

    nc = bacc.Bacc(
        "TRN2", target_bir_lowering=False, debug=False, num_swdge_queues=NSWQ
    )

    # ---- external I/O ----
    xT_ext = nc.dram_tensor("xT", [F_IN, n], BF16, kind="ExternalInput")
    xTo_ext = nc.dram_tensor("xTown", [F_IN, npc], BF16, kind="ExternalInput")
    wh_ext = nc.dram_tensor("W_heads", [HEADS, F_IN, HID], F32, kind="ExternalInput")
    ah_ext = nc.dram_tensor("a_heads", [HEADS, 2 * HID], F32, kind="ExternalInput")
    wo_ext = nc.dram_tensor("W_out", [HEADS * HID, OUT], F32, kind="ExternalInput")
    ao_ext = nc.dram_tensor("a_out", [2 * OUT], F32, kind="ExternalInput")
    NB = cfg.nb
    idx_d16 = nc.dram_tensor("idx_d16", [NB, 128, MBG * 8], I16, kind="ExternalInput")
    idx_s16 = nc.dram_tensor("idx_s16", [NB, 128, MBG * 8], I16, kind="ExternalInput")
    if HOSTOH:
        oh16 = nc.dram_tensor(
            "oh16", [NB, 128, MBG * 128], BF16, kind="ExternalInput"
        )
    else:
        idx_srcl = nc.dram_tensor(
            "idx_srcl", [NB, 128, MBG], I32, kind="ExternalInput"
        )
    out_ext = nc.dram_tensor("out", [npc, OUT], F32, kind="ExternalOutput")

    # ---- internal DRAM ----
    tw = nc.dram_tensor("TW", [n + 2, RW], BF16)
    ts1 = nc.dram_tensor("TS1", [npc, 64], F32)
    hcat = nc.dram_tensor("hcat", [npc, HEADS * HID], BF16)
    t2msh = nc.dram_tensor("T2Msh", [npc, R2W], BF16)
    t2s1 = nc.dram_tensor("T2S1", [npc, 64], F32)
    if cfg.cores > 1:
        t2m = nc.dram_tensor("T2M", [n + 2, R2W], BF16, addr_space="Shared")
    else:
        t2m = nc.dram_tensor("T2M", [n + 2, R2W], BF16)
    if dbg:
        dbg_tw = nc.dram_tensor("dbg_tw", [n + 2, RW], BF16, kind="ExternalOutput")
        dbg_ts1 = nc.dram_tensor("dbg_ts1", [npc, 64], F32, kind="ExternalOutput")
        dbg_hcat = nc.dram_tensor(
            "dbg_hcat", [npc, HEADS * HID], BF16, kind="ExternalOutput"
        )
        dbg_t2m = nc.dram_tensor("dbg_t2m", [n + 2, R2W], BF16, kind="ExternalOutput")
        dbg_t2s1 = nc.dram_tensor("dbg_t2s1", [npc, 64], F32, kind="ExternalOutput")

    # SWDGE queue assignment: Tile binds the 8 DMASW sem lanes to SWDGE DMAs
    # round-robin in issue order, and a lane must stay on one queue -- so pick
    # the queue from a global SWDGE-call counter as (c % 8) % NSWQ, which is
    # constant per lane.
    swc = [0]

    def _q():
        qq = (swc[0] % 8) % NSWQ
        swc[0] += 1
        return qq

    with tile.TileContext(nc) as tc, tc.tile_pool(name="const", bufs=1) as cpool:
        with (
            tc.tile_pool(name="psW", bufs=2, space="PSUM") as psW,
            tc.tile_pool(name="sbW", bufs=2) as sbW,
        ):
            # ======== constants ========
            identb = cpool.tile([128, 128], BF16)
            make_identity(nc, identb[:])
            identf = cpool.tile([128, 128], F32)
            make_identity(nc, identf[:])
            if not HOSTOH:
                iota_i = cpool.tile([128, MBG * 128], I32, tag="iota_i")
                nc.gpsimd.iota(
                    iota_i[:], [[0, MBG], [1, 128]], channel_multiplier=0
                )
                iota_t = cpool.tile([128, MBG * 128], BF16)
                nc.vector.tensor_copy(iota_t[:], iota_i[:])

            # ======== wext = [W_all(256) | c2(4) | c1(4)] bf16 on SBUF ======
            wext = cpool.tile([F_IN, HEADS * HID + 2 * HEADS], BF16)
            wtmp = sbW.tile([F_IN, HEADS * HID], F32, tag="wtmp")
            nc.sync.dma_start(
                wtmp[:].rearrange("p (h o) -> p h o", h=HEADS),
                wh_ext[:].rearrange("h f o -> f h o"),
            )
            nc.scalar.copy(wext[:, 0 : HEADS * HID], wtmp[:])
            ps_c = psW.tile([128, 2 * HEADS], F32, tag="psc")
            for h in range(HEADS):
                wh_t = sbW.tile([F_IN, HID], F32, tag="wh_t")
                nc.sync.dma_start(wh_t[:], wh_ext[h])
                ps_w = psW.tile([HID, F_IN], F32, tag="psw")
                nc.tensor.transpose(ps_w[:], wh_t[:], identf[:])
                whT = sbW.tile([HID, F_IN], F32, tag="whT")
                nc.vector.tensor_copy(whT[:], ps_w[:])
                acol = sbW.tile([HID, 2], F32, tag="acol")
                nc.sync.dma_start(
                    acol[:], ah_ext[h : h + 1, :].rearrange("1 (t o) -> o t", t=2)
                )
                nc.tensor.matmul(
                    ps_c[:, 2 * h : 2 * h + 2], whT[:], acol[:], start=True, stop=True
                )
            nc.vector.tensor_copy(
                wext[:, HEADS * HID : HEADS * HID + HEADS], ps_c[:, 1 : 2 * HEADS : 2]
            )
            nc.vector.tensor_copy(
                wext[:, HEADS * HID + HEADS :], ps_c[:, 0 : 2 * HEADS : 2]
            )

            # ======== sentinel rows (els 0:272 covered; pads unread) ========
            sent = sbW.tile([1, 264], BF16, tag="sent")
            nc.vector.memset(sent[:], 0.0)
            nc.vector.memset(sent[:, 256:264].bitcast(F32), NEG)
            nc.sync.dma_start(tw[0:1, 0:264], sent[:])
            nc.sync.dma_start(tw[n + 1 : n + 2, 0:264], sent[:])
            sent3 = sbW.tile([1, R2W], BF16, tag="sent3")
            nc.vector.memset(sent3[:], 0.0)
            nc.vector.memset(sent3[:, 64:72].bitcast(F32), NEG)
            nc.sync.dma_start(t2m[0:1, :], sent3[:])
            nc.sync.dma_start(t2m[n + 1 : n + 2, :], sent3[:])

        # ======== phase A: build TW (all nodes) + TS1 (own nodes) ======
        with (
            tc.tile_pool(name="psA", bufs=4, space="PSUM") as psA,
            tc.tile_pool(name="sbA", bufs=3) as sbA,
        ):
            for _ra in range(reps.get("A", 1)):
                # TS1 (own nodes): s1 = x_own @ c1
                for t0, kk in _chunks(NW):
                    n0 = 128 * t0
                    cols = min(128 * kk, npc - n0)
                    xo_t = sbA.tile([F_IN, TB * 128], BF16, tag="xo_t")
                    nc.sync.dma_start(xo_t[:, :cols], xTo_ext[:, n0 : n0 + cols])
                    os4 = sbA.tile([128, TB * 4], F32, tag="osA")
                    for k in range(kk):
                        ps_s = psA.tile([128, HEADS], F32, tag="psA_s")
                        nc.tensor.matmul(
                            ps_s[:],
                            xo_t[:, 128 * k : 128 * (k + 1)],
                            wext[:, HEADS * HID + HEADS :],
                            start=True,
                            stop=True,
                        )
                        nc.vector.tensor_copy(os4[:, 4 * k : 4 * k + 4], ps_s[:])
                    full = min(kk, (npc - n0) // 128)
                    if full:
                        nc.sync.dma_start(
                            ts1[n0 : n0 + 128 * full, 0:4].rearrange(
                                "(k p) c -> p k c", p=128
                            ),
                            os4[:, : 4 * full].rearrange("p (k c) -> p k c", c=4),
                        )
                    if full < kk and npc - n0 - 128 * full > 0:
                        rem = npc - n0 - 128 * full
                        nc.sync.dma_start(
                            ts1[n0 + 128 * full : n0 + 128 * full + rem, 0:4],
                            os4[:rem, 4 * full : 4 * full + 4],
                        )
                for t0, kk in _chunks(NT1):
                    n0 = 128 * t0
                    cols = min(128 * kk, n - n0)
                    xT_t = sbA.tile([F_IN, TB * 128], BF16, tag="xT_t")
                    nc.sync.dma_start(xT_t[:, :cols], xT_ext[:, n0 : n0 + cols])
                    ot = sbA.tile([128, TB * 264], BF16, tag="otA")
                    for k in range(kk):
                        ps_o = psA.tile([128, 264], F32, tag="psA_o")
                        nc.tensor.matmul(
                            ps_o[:],
                            xT_t[:, 128 * k : 128 * (k + 1)],
                            wext[:],
                            start=True,
                            stop=True,
                        )
                        nc.scalar.copy(ot[:, 264 * k : 264 * k + 256], ps_o[:, 0:256])
                        nc.vector.tensor_copy(
                            ot[:, 264 * k + 256 : 264 * k + 264].bitcast(F32),
                            ps_o[:, 256:260],
                        )
                    # write complete 128-row tiles in one DMA; clamp remainder
                    full = min(kk, (n - n0) // 128)
                    if full:
                        nc.sync.dma_start(
                            tw[1 + n0 : 1 + n0 + 128 * full, 0:264].rearrange(
                                "(k p) c -> p k c", p=128
                            ),
                            ot[:, : 264 * full].rearrange("p (k c) -> p k c", c=264),
                        )
                    if full < kk and n - n0 - 128 * full > 0:
                        rem = n - n0 - 128 * full
                        nc.sync.dma_start(
                            tw[1 + n0 + 128 * full : 1 + n0 + 128 * full + rem, 0:264],
                            ot[:rem, 264 * full : 264 * full + 264],
                        )

        # ======== phase B: layer-1 edge processing ========
        twh = tw[HALF + 1 :, :] if ANYB else None
        with (
            tc.tile_pool(name="psB", bufs=4, space="PSUM") as psB,
            tc.tile_pool(name="sbB", bufs=2) as sbB,
            tc.tile_pool(name="sbBi", bufs=2) as sbBi,
        ):
            for _rb in range(reps.get("B", 1)):
                for bi, bt in enumerate(cfg.batches):
                    w0, kw, BG, SA = bt["w0"], bt["kw"], bt["bg"], bt["sa"]
                    i16d = sbBi.tile([128, MBG * 8], I16, tag="i16d")
                    nc.sync.dma_start(i16d[:, : BG * 8], idx_d16[bi, :, : BG * 8])
                    i16s = sbBi.tile([128, MBG * 8], I16, tag="i16s")
                    nc.sync.dma_start(i16s[:, : BG * 8], idx_s16[bi, :, : BG * 8])
                    if HOSTOH:
                        oh = sbBi.tile([128, MBG * 128], BF16, tag="oh")
                        nc.sync.dma_start(
                            oh[:, : BG * 128], oh16[bi, :, : BG * 128]
                        )
                    else:
                        srcl = sbBi.tile([128, MBG], I32, tag="srcl")
                        nc.sync.dma_start(srcl[:, :BG], idx_srcl[bi, :, :BG])
                        srclf = sbBi.tile([128, MBG], BF16, tag="srclf")
                        nc.vector.tensor_copy(srclf[:, :BG], srcl[:, :BG])
                        oh = sbB.tile([128, MBG * 128], BF16, tag="oh")
                        nc.vector.tensor_tensor(
                            out=oh[:, : BG * 128].rearrange(
                                "p (g j) -> p g j", j=128
                            ),
                            in0=srclf[:, :BG]
                            .unsqueeze(2)
                            .to_broadcast([128, BG, 128]),
                            in1=iota_t[:, : BG * 128].rearrange(
                                "p (g j) -> p g j", j=128
                            ),
                            op=ALU.is_equal,
                        )

                    g_t = sbB.tile([128, MBG * RW], BF16, tag="g_t")
                    for c0, q in _gcalls(0, SA):
                        nc.gpsimd.dma_gather(
                            g_t[:, c0 * RW : (c0 + q) * RW].rearrange(
                                "p (k e) -> p k e", e=RW
                            ),
                            tw[:],
                            i16d[:, c0 * 8 : (c0 + q) * 8],
                            q * 128,
                            q * 128,
                            RW,
                            queue_num=_q(),
                        )
                    for c0, q in _gcalls(SA, BG):
                        nc.gpsimd.dma_gather(
                            g_t[:, c0 * RW : (c0 + q) * RW].rearrange(
                                "p (k e) -> p k e", e=RW
                            ),
                            twh,
                            i16d[:, c0 * 8 : (c0 + q) * 8],
                            q * 128,
                            q * 128,
                            RW,
                            queue_num=_q(),
                        )
                    s1e = sbB.tile([128, MBG * 64], F32, tag="s1e")
                    for c0, q in _gcalls(0, BG):
                        nc.gpsimd.dma_gather(
                            s1e[:, c0 * 64 : (c0 + q) * 64].rearrange(
                                "p (k e) -> p k e", e=64
                            ),
                            ts1[:],
                            i16s[:, c0 * 8 : (c0 + q) * 8],
                            q * 128,
                            q * 128,
                            64,
                            queue_num=_q(),
                        )

                    # batch-wide: e = lrelu(s1 + s2); ex = exp(e) -> bf16;
                    # R = [ex*Wh | ex] built in place in g_t
                    g3 = g_t[:, : BG * RW].rearrange("p (g c) -> p g c", c=RW)
                    g3f = (
                        g_t[:, : BG * RW]
                        .bitcast(F32)
                        .rearrange("p (g c) -> p g c", c=192)
                    )
                    s13 = s1e[:, : BG * 64].rearrange("p (g c) -> p g c", c=64)
                    e_t = sbB.tile([128, MBG * HEADS], F32, tag="e_t")
                    nc.vector.tensor_add(
                        e_t[:, : BG * HEADS].rearrange("p (g h) -> p g h", h=HEADS),
                        s13[:, :, 0:HEADS],
                        g3f[:, :, 128:132],
                    )
                    lr_t = sbB.tile([128, MBG * HEADS], F32, tag="lr_t")
                    nc.vector.tensor_scalar_mul(
                        lr_t[:, : BG * HEADS], e_t[:, : BG * HEADS], ALPHA
                    )
                    nc.vector.tensor_tensor(
                        lr_t[:, : BG * HEADS],
                        e_t[:, : BG * HEADS],
                        lr_t[:, : BG * HEADS],
                        op=ALU.max,
                    )
                    ex_b = sbB.tile([128, MBG * HEADS], BF16, tag="ex_b")
                    nc.scalar.activation(
                        ex_b[:, : BG * HEADS], lr_t[:, : BG * HEADS], AF.Exp
                    )
                    ex3 = ex_b[:, : BG * HEADS].rearrange("p (g h) -> p g h", h=HEADS)
                    nc.vector.tensor_tensor(
                        out=g3[:, :, 0 : HEADS * HID].rearrange(
                            "p g (h o) -> p g h o", h=HEADS
                        ),
                        in0=g3[:, :, 0 : HEADS * HID].rearrange(
                            "p g (h o) -> p g h o", h=HEADS
                        ),
                        in1=ex3.unsqueeze(3).to_broadcast([128, BG, HEADS, HID]),
                        op=ALU.mult,
                    )
                    nc.vector.tensor_copy(g3[:, :, 256 : 256 + HEADS], ex3)

                    for kwi in range(kw):
                        w = w0 + kwi
                        wn = min(128, npc - 128 * w)
                        gl = list(
                            range(
                                bt["aoff"][kwi], bt["aoff"][kwi] + bt["kas"][kwi]
                            )
                        ) + list(
                            range(
                                bt["boff"][kwi], bt["boff"][kwi] + bt["kbs"][kwi]
                            )
                        )
                        ps_u = psB.tile([128, 260], F32, tag="ps_u")
                        for i, gg in enumerate(gl):
                            nc.tensor.matmul(
                                ps_u[:],
                                oh[:, gg * 128 : (gg + 1) * 128],
                                g_t[:, gg * RW : gg * RW + 260],
                                start=(i == 0),
                                stop=(i == len(gl) - 1),
                            )

                        r4 = sbB.tile([128, HEADS], F32, tag="r4")
                        nc.vector.tensor_scalar_add(r4[:], ps_u[:, 256:260], 1e-30)
                        nc.vector.reciprocal(r4[:], r4[:])
                        hp = sbB.tile([128, HEADS * HID], BF16, tag="hp")
                        nc.vector.tensor_tensor(
                            out=hp[:].rearrange("p (h o) -> p h o", h=HEADS),
                            in0=ps_u[:, 0 : HEADS * HID].rearrange(
                                "p (h o) -> p h o", h=HEADS
                            ),
                            in1=r4[:].unsqueeze(2).to_broadcast([128, HEADS, HID]),
                            op=ALU.mult,
                        )
                        # elu(x) = relu(x) + (exp(min(x,0)) - 1)
                        t0 = sbB.tile([128, HEADS * HID], BF16, tag="elu_t0")
                        nc.vector.tensor_scalar_min(t0[:], hp[:], 0.0)
                        t0e = sbB.tile([128, HEADS * HID], BF16, tag="elu_t0e")
                        nc.scalar.activation(t0e[:], t0[:], AF.Exp)
                        t1 = sbB.tile([128, HEADS * HID], BF16, tag="elu_t1")
                        nc.scalar.activation(t1[:], hp[:], AF.Relu)
                        he = sbB.tile([128, HEADS * HID], BF16, tag="he")
                        nc.vector.scalar_tensor_tensor(
                            he[:], t0e[:], -1.0, t1[:], ALU.add, ALU.add
                        )
                        nc.sync.dma_start(hcat[128 * w : 128 * w + wn, :], he[:wn, :])

        # ======== phase C: build own T2M / T2S1 shards ========
        with (
            tc.tile_pool(name="psC", bufs=2, space="PSUM") as psC,
            tc.tile_pool(name="sbC", bufs=3) as sbC,
            tc.tile_pool(name="cc", bufs=1) as ccpool,
        ):
            # W2ext chunks [128, 66] bf16 x2 : [W_out | c2o | c1o]
            w2e = []
            for c in range(2):
                w2c = ccpool.tile([128, OUT + 2], BF16, tag=f"w2e{c}")
                wo_t = sbC.tile([128, OUT], F32, tag="wo_t")
                nc.sync.dma_start(wo_t[:], wo_ext[128 * c : 128 * (c + 1), :])
                nc.scalar.copy(w2c[:, 0:OUT], wo_t[:])
                ps_w = psC.tile([OUT, 128], F32, tag="psw2")
                nc.tensor.transpose(ps_w[:], wo_t[:], identf[:])
                woT = sbC.tile([OUT, 128], F32, tag="woT")
                nc.vector.tensor_copy(woT[:], ps_w[:])
                aoc = sbC.tile([OUT, 2], F32, tag="aoc")
                nc.sync.dma_start(
                    aoc[:], ao_ext[:].unsqueeze(0).rearrange("1 (t o) -> o t", t=2)
                )
                ps_c2 = psC.tile([128, 2], F32, tag="psc2")
                nc.tensor.matmul(ps_c2[:], woT[:], aoc[:], start=True, stop=True)
                nc.vector.tensor_copy(w2c[:, OUT : OUT + 1], ps_c2[:, 1:2])
                nc.vector.tensor_copy(w2c[:, OUT + 1 : OUT + 2], ps_c2[:, 0:1])
                w2e.append(w2c)

            for _rc in range(reps.get("C", 1)):
                for t0, kk in _chunks(NW):
                    n0 = 128 * t0
                    rows = min(128 * kk, npc - n0)
                    full = min(kk, (npc - n0) // 128)
                    ht4 = sbC.tile([128, TB * HEADS * HID], BF16, tag="ht4")
                    if full:
                        nc.sync.dma_start(
                            ht4[:, : 256 * full].rearrange("p (k c) -> p k c", c=256),
                            hcat[n0 : n0 + 128 * full, :].rearrange(
                                "(k p) c -> p k c", p=128
                            ),
                        )
                    if full < kk:
                        rem = npc - n0 - 128 * full
                        nc.sync.dma_start(
                            ht4[:rem, 256 * full : 256 * full + 256],
                            hcat[n0 + 128 * full : npc, :],
                        )
                    ot = sbC.tile([128, TB * 66], BF16, tag="otC")
                    os4 = sbC.tile([128, TB], F32, tag="osC")
                    for k in range(kk):
                        ps_o = psC.tile([128, OUT + 2], F32, tag="psC_o")
                        for c in range(2):
                            ps_t = psC.tile([128, 128], BF16, tag="psC_t")
                            nc.tensor.transpose(
                                ps_t[:],
                                ht4[:, 256 * k + 128 * c : 256 * k + 128 * (c + 1)],
                                identb[:],
                            )
                            hT = sbC.tile([128, 128], BF16, tag="hT")
                            nc.scalar.copy(hT[:], ps_t[:])
                            nc.tensor.matmul(
                                ps_o[:], hT[:], w2e[c][:], start=(c == 0), stop=(c == 1)
                            )
                        nc.scalar.copy(
                            ot[:, 66 * k : 66 * k + OUT], ps_o[:, 0:OUT]
                        )
                        nc.vector.tensor_copy(
                            ot[:, 66 * k + 64 : 66 * k + 66].bitcast(F32),
                            ps_o[:, OUT : OUT + 1],
                        )
                        nc.vector.tensor_copy(
                            os4[:, k : k + 1], ps_o[:, OUT + 1 : OUT + 2]
                        )
                    if full:
                        nc.sync.dma_start(
                            t2msh[n0 : n0 + 128 * full, 0:66].rearrange(
                                "(k p) c -> p k c", p=128
                            ),
                            ot[:, : 66 * full].rearrange("p (k c) -> p k c", c=66),
                        )
                        nc.sync.dma_start(
                            t2s1[n0 : n0 + 128 * full, 0:1].rearrange(
                                "(k p) c -> p k c", p=128
                            ),
                            os4[:, :full].rearrange("p (k c) -> p k c", c=1),
                        )
                    if full < kk:
                        rem = npc - n0 - 128 * full
                        nc.sync.dma_start(
                            t2msh[n0 + 128 * full : npc, 0:66],
                            ot[:rem, 66 * full : 66 * full + 66],
                        )
                        nc.sync.dma_start(
                            t2s1[n0 + 128 * full : npc, 0:1],
                            os4[:rem, full : full + 1],
                        )

        # ======== phase D: allgather T2M ========
        if cfg.cores > 1 and not mock_d:
            nc.gpsimd.collective_compute(
                "AllGather",
                ALU.bypass,
                replica_groups=[list(range(cfg.cores))],
                ins=[t2msh[:]],
                outs=[t2m[1 : n + 1, :]],
            )
        else:
            nc.sync.dma_start(t2m[1 : npc + 1, :], t2msh[:])

        # ======== phase E: layer-2 edge processing ========
        t2mh = t2m[HALF + 1 :, :] if ANYB else None
        with (
            tc.tile_pool(name="psE", bufs=4, space="PSUM") as psE,
            tc.tile_pool(name="sbE", bufs=2) as sbE,
            tc.tile_pool(name="sbEi", bufs=2) as sbEi,
        ):
            for _re in range(reps.get("E", 1)):
                for bi, bt in enumerate(cfg.batches):
                    w0, kw, BG, SA = bt["w0"], bt["kw"], bt["bg"], bt["sa"]
                    i16d = sbEi.tile([128, MBG * 8], I16, tag="i16d")
                    nc.sync.dma_start(i16d[:, : BG * 8], idx_d16[bi, :, : BG * 8])
                    i16s = sbEi.tile([128, MBG * 8], I16, tag="i16s")
                    nc.sync.dma_start(i16s[:, : BG * 8], idx_s16[bi, :, : BG * 8])
                    if HOSTOH:
                        oh = sbEi.tile([128, MBG * 128], BF16, tag="oh")
                        nc.sync.dma_start(
                            oh[:, : BG * 128], oh16[bi, :, : BG * 128]
                        )
                    else:
                        srcl = sbEi.tile([128, MBG], I32, tag="srcl")
                        nc.sync.dma_start(srcl[:, :BG], idx_srcl[bi, :, :BG])
                        srclf = sbEi.tile([128, MBG], BF16, tag="srclf")
                        nc.vector.tensor_copy(srclf[:, :BG], srcl[:, :BG])
                        oh = sbE.tile([128, MBG * 128], BF16, tag="oh")
                        nc.vector.tensor_tensor(
                            out=oh[:, : BG * 128].rearrange(
                                "p (g j) -> p g j", j=128
                            ),
                            in0=srclf[:, :BG]
                            .unsqueeze(2)
                            .to_broadcast([128, BG, 128]),
                            in1=iota_t[:, : BG * 128].rearrange(
                                "p (g j) -> p g j", j=128
                            ),
                            op=ALU.is_equal,
                        )

                    g2 = sbE.tile([128, MBG * R2W], BF16, tag="g_t2")
                    for c0, q in _gcalls(0, SA):
                        nc.gpsimd.dma_gather(
                            g2[:, c0 * R2W : (c0 + q) * R2W].rearrange(
                                "p (k e) -> p k e", e=R2W
                            ),
                            t2m[:],
                            i16d[:, c0 * 8 : (c0 + q) * 8],
                            q * 128,
                            q * 128,
                            R2W,
                            queue_num=_q(),
                        )
                    for c0, q in _gcalls(SA, BG):
                        nc.gpsimd.dma_gather(
                            g2[:, c0 * R2W : (c0 + q) * R2W].rearrange(
                                "p (k e) -> p k e", e=R2W
                            ),
                            t2mh,
                            i16d[:, c0 * 8 : (c0 + q) * 8],
                            q * 128,
                            q * 128,
                            R2W,
                            queue_num=_q(),
                        )
                    s1e = sbE.tile([128, MBG * 64], F32, tag="s1e2")
                    for c0, q in _gcalls(0, BG):
                        nc.gpsimd.dma_gather(
                            s1e[:, c0 * 64 : (c0 + q) * 64].rearrange(
                                "p (k e) -> p k e", e=64
                            ),
                            t2s1[:],
                            i16s[:, c0 * 8 : (c0 + q) * 8],
                            q * 128,
                            q * 128,
                            64,
                            queue_num=_q(),
                        )

                    g23 = g2[:, : BG * R2W].rearrange("p (g c) -> p g c", c=R2W)
                    g23f = (
                        g2[:, : BG * R2W]
                        .bitcast(F32)
                        .rearrange("p (g c) -> p g c", c=64)
                    )
                    s13 = s1e[:, : BG * 64].rearrange("p (g c) -> p g c", c=64)
                    e_t = sbE.tile([128, MBG], F32, tag="e_t2")
                    nc.vector.tensor_add(
                        e_t[:, :BG].unsqueeze(2), s13[:, :, 0:1], g23f[:, :, 32:33]
                    )
                    lr_t = sbE.tile([128, MBG], F32, tag="lr_t2")
                    nc.vector.tensor_scalar_mul(lr_t[:, :BG], e_t[:, :BG], ALPHA)
                    nc.vector.tensor_tensor(
                        lr_t[:, :BG], e_t[:, :BG], lr_t[:, :BG], op=ALU.max
                    )
                    ex_b = sbE.tile([128, MBG], BF16, tag="ex_b2")
                    nc.scalar.activation(ex_b[:, :BG], lr_t[:, :BG], AF.Exp)
                    nc.vector.tensor_tensor(
                        out=g23[:, :, 0:OUT],
                        in0=g23[:, :, 0:OUT],
                        in1=ex_b[:, :BG].unsqueeze(2).to_broadcast([128, BG, OUT]),
                        op=ALU.mult,
                    )
                    nc.vector.tensor_copy(
                        g23[:, :, OUT : OUT + 1], ex_b[:, :BG].unsqueeze(2)
                    )

                    for kwi in range(kw):
                        w = w0 + kwi
                        wn = min(128, npc - 128 * w)
                        gl = list(
                            range(
                                bt["aoff"][kwi], bt["aoff"][kwi] + bt["kas"][kwi]
                            )
                        ) + list(
                            range(
                                bt["boff"][kwi], bt["boff"][kwi] + bt["kbs"][kwi]
                            )
                        )
                        ps_u = psE.tile([128, OUT + 1], F32, tag="ps_u2")
                        for i, gg in enumerate(gl):
                            nc.tensor.matmul(
                                ps_u[:],
                                oh[:, gg * 128 : (gg + 1) * 128],
                                g2[:, gg * R2W : gg * R2W + OUT + 1],
                                start=(i == 0),
                                stop=(i == len(gl) - 1),
                            )

                        r1 = sbE.tile([128, 1], F32, tag="r12")
                        nc.vector.tensor_scalar_add(
                            r1[:], ps_u[:, OUT : OUT + 1], 1e-30
                        )
                        nc.vector.reciprocal(r1[:], r1[:])
                        op_t = sbE.tile([128, OUT], F32, tag="op_t")
                        nc.vector.tensor_tensor(
                            out=op_t[:],
                            in0=ps_u[:, 0:OUT],
                            in1=r1[:].to_broadcast([128, OUT]),
                            op=ALU.mult,
                        )
                        t0 = sbE.tile([128, OUT], F32, tag="elu2_t0")
                        nc.vector.tensor_scalar_min(t0[:], op_t[:], 0.0)
                        t0e = sbE.tile([128, OUT], F32, tag="elu2_t0e")
                        nc.scalar.activation(t0e[:], t0[:], AF.Exp)
                        t1 = sbE.tile([128, OUT], F32, tag="elu2_t1")
                        nc.scalar.activation(t1[:], op_t[:], AF.Relu)
                        oe = sbE.tile([128, OUT], F32, tag="oe")
                        nc.vector.scalar_tensor_tensor(
                            oe[:], t0e[:], -1.0, t1[:], ALU.add, ALU.add
                        )
                        nc.sync.dma_start(
                            out_ext[128 * w : 128 * w + wn, :], oe[:wn, :]
                        )

        if dbg:
            nc.sync.dma_start(dbg_tw[:], tw[:])
            nc.sync.dma_start(dbg_ts1[:], ts1[:])
            nc.sync.dma_start(dbg_hcat[:], hcat[:])
            nc.sync.dma_start(dbg_t2m[:], t2m[:])
            nc.sync.dma_start(dbg_t2s1[:], t2s1[:])

    nc.compile()
    return nc


# ---------------------------------------------------------------------------
# Host-side preparation and execution
# ---------------------------------------------------------------------------


def _pack16_slots(slot_vals, nw, g):
    """slot_vals [NW, G*128] in slot order j -> [NW, 128, G*8] int16 layout:
    idx j at [16*r + j%16, j//16], replicated for r in 0..7."""
    w = slot_vals.reshape(nw, g * 8, 16)  # [NW, j//16, j%16]
    w = np.swapaxes(w, 1, 2)  # [NW, 16, G*8]
    return np.ascontiguousarray(np.tile(w, (1, 8, 1)).astype(np.int16))


def section_sizes(n, cores, edges):
    """Per-window-index max A/B-section group counts over cores."""
    npc = n // cores
    nw = (npc + 127) // 128
    src = np.asarray(edges[0], dtype=np.int64)
    dst = np.asarray(edges[1], dtype=np.int64)
    order = np.argsort(src, kind="stable")
    ssrc, sdst = src[order], dst[order]
    kaw = np.zeros(nw, dtype=np.int64)
    kbw = np.zeros(nw, dtype=np.int64)
    for k in range(cores):
        for w in range(nw):
            lo = k * npc + 128 * w
            hi = min(lo + 128, (k + 1) * npc)
            s0, s1 = np.searchsorted(ssrc, [lo, hi])
            d = sdst[s0:s1]
            ca = int((d < HALF).sum())
            cb = int(len(d) - ca)
            kaw[w] = max(kaw[w], max(1, -(-ca // 128)))
            kbw[w] = max(kbw[w], -(-cb // 128))
    if n + 1 > HALF:
        kbw = np.maximum(kbw, 1)
    return kaw, kbw


def prepare_inputs(cfg: Cfg, x, edges, W_heads, a_heads, W_out, a_out):
    """Build per-core input maps. Pure layout/index manipulation.

    Gather-batch layout: windows are grouped WB at a time; within a batch the
    slot/group order is [A-sec w0 | A-sec w1 | ... | B-sec w0 | B-sec w1 ...]
    with per-window group counts cfg.kaw/kbw, so each section is one
    contiguous span of dma_gather calls over all kw windows."""
    n, cores, npc, NW = cfg.n, cfg.cores, cfg.npc, cfg.nw
    NB, MBG = cfg.nb, cfg.maxbg
    src = np.asarray(edges[0], dtype=np.int64)
    dst = np.asarray(edges[1], dtype=np.int64)
    order = np.argsort(src, kind="stable")
    ssrc = src[order]
    sdst = dst[order]

    xT = np.ascontiguousarray(np.asarray(x, np.float32).T.astype(BF_NP))

    common = dict(
        xT=xT,
        W_heads=np.asarray(W_heads, np.float32),
        a_heads=np.asarray(a_heads, np.float32),
        W_out=np.asarray(W_out, np.float32),
        a_out=np.asarray(a_out, np.float32),
    )

    ar128 = np.arange(128, dtype=np.int32)
    in_maps = []
    for k in range(cores):
        vd = np.zeros((NB, MBG * 128), dtype=np.int64)
        vs = np.zeros((NB, MBG * 128), dtype=np.int64)
        if HOSTOH:
            oh = np.zeros((NB, 128, MBG * 128), dtype=BF_NP)
        else:
            vl = np.zeros((NB, 128, MBG), dtype=np.int32)
        for bi, bt in enumerate(cfg.batches):
            w0, kw = bt["w0"], bt["kw"]
            dA, sA, wA = [], [], []
            dB, sB, wB = [], [], []
            for kwi in range(kw):
                w = w0 + kwi
                ka, kb = bt["kas"][kwi], bt["kbs"][kwi]
                lo = k * npc + 128 * w
                hi = min(lo + 128, (k + 1) * npc)
                s0, s1 = np.searchsorted(ssrc, [lo, hi])
                d, sr = sdst[s0:s1], ssrc[s0:s1]
                selA = d < HALF
                da, sa = d[selA], sr[selA]
                db, sb = d[~selA], sr[~selA]
                assert len(da) <= 128 * ka and len(db) <= 128 * kb
                rowA = np.zeros(128 * ka, dtype=np.int64)
                rowA[: len(da)] = da + 1
                dA.append(rowA)
                rowB = np.full(128 * kb, n + 1 - (HALF + 1), dtype=np.int64)
                rowB[: len(db)] = db + 1 - (HALF + 1)
                dB.append(rowB)
                sa_l = np.zeros(128 * ka, dtype=np.int64)
                sa_l[: len(sa)] = sa - k * npc
                sA.append(sa_l)
                sb_l = np.zeros(128 * kb, dtype=np.int64)
                sb_l[: len(sb)] = sb - k * npc
                sB.append(sb_l)
                wa = np.zeros(128 * ka, dtype=np.int32)
                wa[: len(sa)] = (sa - lo).astype(np.int32)
                wA.append(wa)
                wb_ = np.zeros(128 * kb, dtype=np.int32)
                wb_[: len(sb)] = (sb - lo).astype(np.int32)
                wB.append(wb_)
            vdb = np.concatenate(dA + dB)
            vsb = np.concatenate(sA + sB)
            vd[bi, : len(vdb)] = vdb
            vs[bi, : len(vsb)] = vsb
            wl = np.concatenate(wA + wB)  # [BG*128] slot-major (g, p)
            wlT = wl.reshape(-1, 128).T  # [128, BG]
            if HOSTOH:
                ohb = (wlT[:, :, None] == ar128[None, None, :]).astype(BF_NP)
                oh[bi, :, : ohb.shape[1] * 128] = ohb.reshape(128, -1)
            else:
                vl[bi, :, : wlT.shape[1]] = wlT
        in_maps.append(
            dict(
                common,
                xTown=np.ascontiguousarray(xT[:, k * npc : (k + 1) * npc]),
                idx_d16=_pack16_slots(vd, NB, MBG),
                idx_s16=_pack16_slots(vs, NB, MBG),
                **({"oh16": oh} if HOSTOH else {"idx_srcl": vl}),
            )
        )
    return in_maps


_NC_CACHE = {}


def get_nc(cfg: Cfg):
    if cfg.key not in _NC_CACHE:
        _NC_CACHE[cfg.key] = build_nc(cfg)
    return _NC_CACHE[cfg.key]


def make_cfg(n, cores, edges):
    kaw, kbw = section_sizes(n, cores, edges)
    if UNIKA:
        kaw = np.full_like(kaw, kaw.max())
        kbw = np.full_like(kbw, kbw.max())
    return Cfg(n, cores, kaw, kbw)


def run(inputs, trace=False, **spmd_kwargs):
    from concourse.bass_utils import run_bass_kernel_spmd

    x = np.asarray(inputs["x"], np.float32)
    edges = np.asarray(inputs["edges"])
    cfg = make_cfg(N, CORES, edges)
    nc = get_nc(cfg)
    in_maps = prepare_inputs(
        cfg,
        x,
        edges,
        inputs["W_heads"],
        inputs["a_heads"],
        inputs["W_out"],
        inputs["a_out"],
    )
    res = run_bass_kernel_spmd(
        nc, in_maps, core_ids=list(range(CORES)), trace=trace, **spmd_kwargs
    )
    out = np.concatenate([r["out"] for r in res.results], axis=0)
    return out, res


def kernel(**inputs):
    return run(inputs)[0]


# revision 26
# speedup vs baseline: 1.5582x; 1.5582x over previous
"""Trainium2 Bass kernel for 2-layer multi-head GAT (nn_GAT_38551626449703).

Strategy (8 NeuronCores, SPMD):
  - Nodes are partitioned uniformly: core k owns nodes [k*NPC, (k+1)*NPC).
  - Edges are sharded by OWNER OF src (softmax groups by src stay core-local).
  - Per core, edges are grouped into 128-node windows; window w has
    kaw[w]+kbw[w] groups of 128 edge-slots (per-window counts = max over
    cores, ~9% fewer slots than a uniform max), split into an A-section
    (dst < 32767) and a B-section (dst >= 32767) so table rows fit int16
    indices for dma_gather. Windows are processed WB=3 at a time; each
    section is a contiguous span of <=768-index gather calls (the SWDGE
    descriptor ring holds 1024 descriptors per queue).
  - Gather tables (dma_gather rows must be 256B-multiples):
      TW   [N+2, 384] bf16: els 0:256 Wh (4 heads), els 256:264 = s2 as 4xf32,
                            rest pad. Rows n+1; rows 0 / N+1 are sentinels
                            with s2 = -1e30 (768B rows).
      TS1  [NPC, 64] f32:  els 0:4 = s1 (by src, core-local; 256B rows)
      T2M  [N+2, 128] bf16: els 0:64 Wh2, els 64:66 = s2o as 1xf32 (256B rows)
      T2S1 [NPC, 64] f32:  el 0 = s1o
  - Pad slots gather sentinel rows (s2 = -1e30 -> exp(e) == 0 exactly).
  - Per batch: gathers, then e = lrelu(s1+s2), ex = exp(e) -> bf16 (Act
    engine), R = [ex*Wh | ex] bf16 in place; per window: host-precomputed
    one-hot(src) bf16 matmuls accumulate [u | denom] per node in PSUM
    (1 cyc/row), then h' = u/denom, ELU (Act engine assisted).
  - Between layers only the compact T2M shard (1.6 MB bf16) is AllGathered.
  - Outputs (rows for owned nodes) are concatenated on the host.
"""

import os
import sys

import numpy as np

sys.path.insert(0, "/opt/trn_rl_repo")

import ml_dtypes  # noqa: E402

import concourse.bacc as bacc  # noqa: E402
import concourse.bass as bass  # noqa: E402
import concourse.tile as tile  # noqa: E402
from concourse import mybir  # noqa: E402
from concourse.masks import make_identity  # noqa: E402

F32 = mybir.dt.float32
BF16 = mybir.dt.bfloat16
I32 = mybir.dt.int32
I16 = mybir.dt.int16
AF = mybir.ActivationFunctionType
ALU = mybir.AluOpType
BF_NP = ml_dtypes.bfloat16

# Problem constants
N = 50000
E = 800000
F_IN = 128
HID = 64
HEADS = 4
OUT = 64
ALPHA = 0.2
CORES = 8

NEG = -1.0e30  # sentinel s2 -> exp(lrelu(s1+NEG)) == 0.0 in f32
HALF = 32767  # dst < HALF -> A section (table row dst+1 <= 32767)
NSWQ = int(os.environ.get("GAT_NSWQ", "4"))  # SWDGE queues
HOSTOH = bool(int(os.environ.get("GAT_HOSTOH", "1")))  # host-built one-hot
QG = int(os.environ.get("GAT_QG", "6"))  # groups per dma_gather call
UNIKA = bool(int(os.environ.get("GAT_UNIKA", "0")))  # uniform ka/kb
# SWDGE descriptor ring is dynamic_dma_scratch_size/16 = 1024 descs per
# queue; a single gather call must stay well under that (QG*128 <= 768).

# Tile assigns the 8 DMASW completion-sem lanes round-robin over Pool-engine
# DMAs regardless of SWDGE queue, but a lane must stay on ONE queue (ucode
# constraint; violations -> corrupted sync / device crash). With NSWQ > 1 we
# partition the lanes: queue q owns lanes [q*8//NSWQ, (q+1)*8//NSWQ).
if NSWQ > 1:
    import concourse.bass_isa as _bass_isa
    import concourse.tile_sem_assignment as _tsa

    _orig_assign_tick = _tsa.TileClockTick._assign_tick

    def _lane_partitioned_assign_tick(self, inst):
        if (
            isinstance(inst, _tsa.DMAInst)
            and inst.engine == mybir.EngineType.Pool
            and not isinstance(inst, _bass_isa.UserSyncedRemoteDMADescs)
        ):
            qn = getattr(inst, "queue_num", 0) or 0
            per = getattr(self, "_q_lane_ctr", None)
            if per is None:
                per = self._q_lane_ctr = {}
            lanes = 8 // NSWQ
            c = per.get(qn, 0)
            per[qn] = c + 1
            self.next_sw_dma_idx = qn * lanes + (c % lanes)
        return _orig_assign_tick(self, inst)

    if _tsa.TileClockTick._assign_tick is not _lane_partitioned_assign_tick:
        _tsa.TileClockTick._assign_tick = _lane_partitioned_assign_tick

RW = 384  # TW row elements (bf16) = 768B
R2W = 128  # T2M row elements (bf16) = 256B
TB = 4  # tiles per batched DMA (phases A/C)
WB = 3  # windows per gather batch (phases B/E)


class Cfg:
    def __init__(self, n, cores, kaw, kbw):
        assert n % cores == 0
        self.n = n
        self.cores = cores
        self.npc = n // cores
        self.nw = (self.npc + 127) // 128  # windows per core
        self.kaw = [int(v) for v in kaw]  # A-section groups, per window
        self.kbw = [int(v) for v in kbw]  # B-section groups, per window
        assert len(self.kaw) == self.nw and len(self.kbw) == self.nw
        self.g = max(a + b for a, b in zip(self.kaw, self.kbw))
        self.nb = (self.nw + WB - 1) // WB  # gather batches
        # per-batch layout: [A(w0)..A(wk) | B(w0)..B(wk)] group spans
        self.batches = []
        for w0, kw in _chunks(self.nw, WB):
            kas = self.kaw[w0 : w0 + kw]
            kbs = self.kbw[w0 : w0 + kw]
            sa = sum(kas)
            aoff = [sum(kas[:i]) for i in range(kw)]
            boff = [sa + sum(kbs[:i]) for i in range(kw)]
            self.batches.append(
                dict(w0=w0, kw=kw, kas=kas, kbs=kbs, aoff=aoff, boff=boff,
                     sa=sa, bg=sa + sum(kbs))
            )
        self.maxbg = max(b["bg"] for b in self.batches)
        self.key = (n, cores, tuple(self.kaw), tuple(self.kbw))


def _chunks(nt, step=TB):
    """[(t0, ntiles), ...] batches of `step` tiles."""
    out = []
    t = 0
    while t < nt:
        k = min(step, nt - t)
        out.append((t, k))
        t += k
    return out


def _gcalls(g0, g1):
    """Split groups [g0, g1) into dma_gather calls of <= QG groups."""
    out = []
    g = g0
    while g < g1:
        q = min(QG, g1 - g)
        out.append((g, q))
        g += q
    return out


def build_nc(cfg: Cfg, dbg: bool = False, reps=None, mock_d: bool = False):
    """Build the SPMD Bass program (one program, runs on all cores).

    mock_d: replace the AllGather with a local DMA (for single-core
    cost-model simulation only; wrong results on real multi-core runs)."""
    reps = reps or {}
    n, npc, NW = cfg.n, cfg.npc, cfg.nw
    MBG = cfg.maxbg
    ANYB = max(cfg.kbw) > 0
    NT1 = (n + 127) // 128

    nc = bacc.Bacc(
        "TRN2", target_bir_lowering=False, debug=False, num_swdge_queues=NSWQ
    )

    # ---- external I/O ----
    xT_ext = nc.dram_tensor("xT", [F_IN, n], BF16, kind="ExternalInput")
    xTo_ext = nc.dram_tensor("xTown", [F_IN, npc], BF16, kind="ExternalInput")
    wh_ext = nc.dram_tensor("W_heads", [HEADS, F_IN, HID], F32, kind="ExternalInput")
    ah_ext = nc.dram_tensor("a_heads", [HEADS, 2 * HID], F32, kind="ExternalInput")
    wo_ext = nc.dram_tensor("W_out", [HEADS * HID, OUT], F32, kind="ExternalInput")
    ao_ext = nc.dram_tensor("a_out", [2 * OUT], F32, kind="ExternalInput")
    NB = cfg.nb
    idx_d16 = nc.dram_tensor("idx_d16", [NB, 128, MBG * 8], I16, kind="ExternalInput")
    idx_s16 = nc.dram_tensor("idx_s16", [NB, 128, MBG * 8], I16, kind="ExternalInput")
    if HOSTOH:
        oh16 = nc.dram_tensor(
            "oh16", [NB, 128, MBG * 128], BF16, kind="ExternalInput"
        )
    else:
        idx_srcl = nc.dram_tensor(
            "idx_srcl", [NB, 128, MBG], I32, kind="ExternalInput"
        )
    out_ext = nc.dram_tensor("out", [npc, OUT], F32, kind="ExternalOutput")

    # ---- internal DRAM ----
    tw = nc.dram_tensor("TW", [n + 2, RW], BF16)
    ts1 = nc.dram_tensor("TS1", [npc, 64], F32)
    hcat = nc.dram_tensor("hcat", [npc, HEADS * HID], BF16)
    t2msh = nc.dram_tensor("T2Msh", [npc, R2W], BF16)
    t2s1 = nc.dram_tensor("T2S1", [npc, 64], F32)
    if cfg.cores > 1:
        t2m = nc.dram_tensor("T2M", [n + 2, R2W], BF16, addr_space="Shared")
    else:
        t2m = nc.dram_tensor("T2M", [n + 2, R2W], BF16)
    if dbg:
        dbg_tw = nc.dram_tensor("dbg_tw", [n + 2, RW], BF16, kind="ExternalOutput")
        dbg_ts1 = nc.dram_tensor("dbg_ts1", [npc, 64], F32, kind="ExternalOutput")
        dbg_hcat = nc.dram_tensor(
            "dbg_hcat", [npc, HEADS * HID], BF16, kind="ExternalOutput"
        )
        dbg_t2m = nc.dram_tensor("dbg_t2m", [n + 2, R2W], BF16, kind="ExternalOutput")
        dbg_t2s1 = nc.dram_tensor("dbg_t2s1", [npc, 64], F32, kind="ExternalOutput")

    # SWDGE queue assignment: Tile binds the 8 DMASW sem lanes to SWDGE DMAs
    # round-robin in issue order, and a lane must stay on one queue -- so pick
    # the queue from a global SWDGE-call counter as (c % 8) % NSWQ, which is
    # constant per lane.
    swc = [0]

    def _q():
        qq = (swc[0] % 8) % NSWQ
        swc[0] += 1
        return qq

    with tile.TileContext(nc) as tc, tc.tile_pool(name="const", bufs=1) as cpool:
        with (
            tc.tile_pool(name="psW", bufs=2, space="PSUM") as psW,
            tc.tile_pool(name="sbW", bufs=2) as sbW,
        ):
            # ======== constants ========
            identb = cpool.tile([128, 128], BF16)
            make_identity(nc, identb[:])
            identf = cpool.tile([128, 128], F32)
            make_identity(nc, identf[:])
            if not HOSTOH:
                iota_i = cpool.tile([128, MBG * 128], I32, tag="iota_i")
                nc.gpsimd.iota(
                    iota_i[:], [[0, MBG], [1, 128]], channel_multiplier=0
                )
                iota_t = cpool.tile([128, MBG * 128], BF16)
                nc.vector.tensor_copy(iota_t[:], iota_i[:])

            # ======== wext = [W_all(256) | c2(4) | c1(4)] bf16 on SBUF ======
            wext = cpool.tile([F_IN, HEADS * HID + 2 * HEADS], BF16)
            wtmp = sbW.tile([F_IN, HEADS * HID], F32, tag="wtmp")
            nc.sync.dma_start(
                wtmp[:].rearrange("p (h o) -> p h o", h=HEADS),
                wh_ext[:].rearrange("h f o -> f h o"),
            )
            nc.scalar.copy(wext[:, 0 : HEADS * HID], wtmp[:])
            ps_c = psW.tile([128, 2 * HEADS], F32, tag="psc")
            for h in range(HEADS):
                wh_t = sbW.tile([F_IN, HID], F32, tag="wh_t")
                nc.sync.dma_start(wh_t[:], wh_ext[h])
                ps_w = psW.tile([HID, F_IN], F32, tag="psw")
                nc.tensor.transpose(ps_w[:], wh_t[:], identf[:])
                whT = sbW.tile([HID, F_IN], F32, tag="whT")
                nc.vector.tensor_copy(whT[:], ps_w[:])
                acol = sbW.tile([HID, 2], F32, tag="acol")
                nc.sync.dma_start(
                    acol[:], ah_ext[h : h + 1, :].rearrange("1 (t o) -> o t", t=2)
                )
                nc.tensor.matmul(
                    ps_c[:, 2 * h : 2 * h + 2], whT[:], acol[:], start=True, stop=True
                )
            nc.vector.tensor_copy(
                wext[:, HEADS * HID : HEADS * HID + HEADS], ps_c[:, 1 : 2 * HEADS : 2]
            )
            nc.vector.tensor_copy(
                wext[:, HEADS * HID + HEADS :], ps_c[:, 0 : 2 * HEADS : 2]
            )

            # ======== sentinel rows (els 0:272 covered; pads unread) ========
            sent = sbW.tile([1, 264], BF16, tag="sent")
            nc.vector.memset(sent[:], 0.0)
            nc.vector.memset(sent[:, 256:264].bitcast(F32), NEG)
            nc.sync.dma_start(tw[0:1, 0:264], sent[:])
            nc.sync.dma_start(tw[n + 1 : n + 2, 0:264], sent[:])
            sent3 = sbW.tile([1, R2W], BF16, tag="sent3")
            nc.vector.memset(sent3[:], 0.0)
            nc.vector.memset(sent3[:, 64:72].bitcast(F32), NEG)
            nc.sync.dma_start(t2m[0:1, :], sent3[:])
            nc.sync.dma_start(t2m[n + 1 : n + 2, :], sent3[:])

        # ======== phase A: build TW (all nodes) + TS1 (own nodes) ======
        with (
            tc.tile_pool(name="psA", bufs=4, space="PSUM") as psA,
            tc.tile_pool(name="sbA", bufs=3) as sbA,
        ):
            for _ra in range(reps.get("A", 1)):
                # TS1 (own nodes): s1 = x_own @ c1
                for t0, kk in _chunks(NW):
                    n0 = 128 * t0
                    cols = min(128 * kk, npc - n0)
                    xo_t = sbA.tile([F_IN, TB * 128], BF16, tag="xo_t")
                    nc.sync.dma_start(xo_t[:, :cols], xTo_ext[:, n0 : n0 + cols])
                    os4 = sbA.tile([128, TB * 4], F32, tag="osA")
                    for k in range(kk):
                        ps_s = psA.tile([128, HEADS], F32, tag="psA_s")
                        nc.tensor.matmul(
                            ps_s[:],
                            xo_t[:, 128 * k : 128 * (k + 1)],
                            wext[:, HEADS * HID + HEADS :],
                            start=True,
                            stop=True,
                        )
                        nc.vector.tensor_copy(os4[:, 4 * k : 4 * k + 4], ps_s[:])
                    full = min(kk, (npc - n0) // 128)
                    if full:
                        nc.sync.dma_start(
                            ts1[n0 : n0 + 128 * full, 0:4].rearrange(
                                "(k p) c -> p k c", p=128
                            ),
                            os4[:, : 4 * full].rearrange("p (k c) -> p k c", c=4),
                        )
                    if full < kk and npc - n0 - 128 * full > 0:
                        rem = npc - n0 - 128 * full
                        nc.sync.dma_start(
                            ts1[n0 + 128 * full : n0 + 128 * full + rem, 0:4],
                            os4[:rem, 4 * full : 4 * full + 4],
                        )
                for t0, kk in _chunks(NT1):
                    n0 = 128 * t0
                    cols = min(128 * kk, n - n0)
                    xT_t = sbA.tile([F_IN, TB * 128], BF16, tag="xT_t")
                    nc.sync.dma_start(xT_t[:, :cols], xT_ext[:, n0 : n0 + cols])
                    ot = sbA.tile([128, TB * 264], BF16, tag="otA")
                    for k in range(kk):
                        ps_o = psA.tile([128, 264], F32, tag="psA_o")
                        nc.tensor.matmul(
                            ps_o[:],
                            xT_t[:, 128 * k : 128 * (k + 1)],
                            wext[:],
                            start=True,
                            stop=True,
                        )
                        nc.scalar.copy(ot[:, 264 * k : 264 * k + 256], ps_o[:, 0:256])
                        nc.vector.tensor_copy(
                            ot[:, 264 * k + 256 : 264 * k + 264].bitcast(F32),
                            ps_o[:, 256:260],
                        )
                    # write complete 128-row tiles in one DMA; clamp remainder
                    full = min(kk, (n - n0) // 128)
                    if full:
                        nc.sync.dma_start(
                            tw[1 + n0 : 1 + n0 + 128 * full, 0:264].rearrange(
                                "(k p) c -> p k c", p=128
                            ),
                            ot[:, : 264 * full].rearrange("p (k c) -> p k c", c=264),
                        )
                    if full < kk and n - n0 - 128 * full > 0:
                        rem = n - n0 - 128 * full
                        nc.sync.dma_start(
                            tw[1 + n0 + 128 * full : 1 + n0 + 128 * full + rem, 0:264],
                            ot[:rem, 264 * full : 264 * full + 264],
                        )

        # ======== phase B: layer-1 edge processing ========
        twh = tw[HALF + 1 :, :] if ANYB else None
        with (
            tc.tile_pool(name="psB", bufs=4, space="PSUM") as psB,
            tc.tile_pool(name="sbB", bufs=2) as sbB,
            tc.tile_pool(name="sbBi", bufs=2) as sbBi,
        ):
            for _rb in range(reps.get("B", 1)):
                for bi, bt in enumerate(cfg.batches):
                    w0, kw, BG, SA = bt["w0"], bt["kw"], bt["bg"], bt["sa"]
                    i16d = sbBi.tile([128, MBG * 8], I16, tag="i16d")
                    nc.sync.dma_start(i16d[:, : BG * 8], idx_d16[bi, :, : BG * 8])
                    i16s = sbBi.tile([128, MBG * 8], I16, tag="i16s")
                    nc.sync.dma_start(i16s[:, : BG * 8], idx_s16[bi, :, : BG * 8])
                    if HOSTOH:
                        oh = sbBi.tile([128, MBG * 128], BF16, tag="oh")
                        nc.sync.dma_start(
                            oh[:, : BG * 128], oh16[bi, :, : BG * 128]
                        )
                    else:
                        srcl = sbBi.tile([128, MBG], I32, tag="srcl")
                        nc.sync.dma_start(srcl[:, :BG], idx_srcl[bi, :, :BG])
                        srclf = sbBi.tile([128, MBG], BF16, tag="srclf")
                        nc.vector.tensor_copy(srclf[:, :BG], srcl[:, :BG])
                        oh = sbB.tile([128, MBG * 128], BF16, tag="oh")
                        nc.vector.tensor_tensor(
                            out=oh[:, : BG * 128].rearrange(
                                "p (g j) -> p g j", j=128
                            ),
                            in0=srclf[:, :BG]
                            .unsqueeze(2)
                            .to_broadcast([128, BG, 128]),
                            in1=iota_t[:, : BG * 128].rearrange(
                                "p (g j) -> p g j", j=128
                            ),
                            op=ALU.is_equal,
                        )

                    g_t = sbB.tile([128, MBG * RW], BF16, tag="g_t")
                    for c0, q in _gcalls(0, SA):
                        nc.gpsimd.dma_gather(
                            g_t[:, c0 * RW : (c0 + q) * RW].rearrange(
                                "p (k e) -> p k e", e=RW
                            ),
                            tw[:],
                            i16d[:, c0 * 8 : (c0 + q) * 8],
                            q * 128,
                            q * 128,
                            RW,
                            queue_num=_q(),
                        )
                    for c0, q in _gcalls(SA, BG):
                        nc.gpsimd.dma_gather(
                            g_t[:, c0 * RW : (c0 + q) * RW].rearrange(
                                "p (k e) -> p k e", e=RW
                            ),
                            twh,
                            i16d[:, c0 * 8 : (c0 + q) * 8],
                            q * 128,
                            q * 128,
                            RW,
                            queue_num=_q(),
                        )
                    s1e = sbB.tile([128, MBG * 64], F32, tag="s1e")
                    for c0, q in _gcalls(0, BG):
                        nc.gpsimd.dma_gather(
                            s1e[:, c0 * 64 : (c0 + q) * 64].rearrange(
                                "p (k e) -> p k e", e=64
                            ),
                            ts1[:],
                            i16s[:, c0 * 8 : (c0 + q) * 8],
                            q * 128,
                            q * 128,
                            64,
                            queue_num=_q(),
                        )

                    # batch-wide: e = lrelu(s1 + s2); ex = exp(e) -> bf16;
                    # R = [ex*Wh | ex] built in place in g_t
                    g3 = g_t[:, : BG * RW].rearrange("p (g c) -> p g c", c=RW)
                    g3f = (
                        g_t[:, : BG * RW]
                        .bitcast(F32)
                        .rearrange("p (g c) -> p g c", c=192)
                    )
                    s13 = s1e[:, : BG * 64].rearrange("p (g c) -> p g c", c=64)
                    e_t = sbB.tile([128, MBG * HEADS], F32, tag="e_t")
                    nc.vector.tensor_add(
                        e_t[:, : BG * HEADS].rearrange("p (g h) -> p g h", h=HEADS),
                        s13[:, :, 0:HEADS],
                        g3f[:, :, 128:132],
                    )
                    lr_t = sbB.tile([128, MBG * HEADS], F32, tag="lr_t")
                    nc.vector.tensor_scalar_mul(
                        lr_t[:, : BG * HEADS], e_t[:, : BG * HEADS], ALPHA
                    )
                    nc.vector.tensor_tensor(
                        lr_t[:, : BG * HEADS],
                        e_t[:, : BG * HEADS],
                        lr_t[:, : BG * HEADS],
                        op=ALU.max,
                    )
                    ex_b = sbB.tile([128, MBG * HEADS], BF16, tag="ex_b")
                    nc.scalar.activation(
                        ex_b[:, : BG * HEADS], lr_t[:, : BG * HEADS], AF.Exp
                    )
                    ex3 = ex_b[:, : BG * HEADS].rearrange("p (g h) -> p g h", h=HEADS)
                    nc.vector.tensor_tensor(
                        out=g3[:, :, 0 : HEADS * HID].rearrange(
                            "p g (h o) -> p g h o", h=HEADS
                        ),
                        in0=g3[:, :, 0 : HEADS * HID].rearrange(
                            "p g (h o) -> p g h o", h=HEADS
                        ),
                        in1=ex3.unsqueeze(3).to_broadcast([128, BG, HEADS, HID]),
                        op=ALU.mult,
                    )
                    nc.vector.tensor_copy(g3[:, :, 256 : 256 + HEADS], ex3)

                    for kwi in range(kw):
                        w = w0 + kwi
                        wn = min(128, npc - 128 * w)
                        gl = list(
                            range(
                                bt["aoff"][kwi], bt["aoff"][kwi] + bt["kas"][kwi]
                            )
                        ) + list(
                            range(
                                bt["boff"][kwi], bt["boff"][kwi] + bt["kbs"][kwi]
                            )
                        )
                        ps_u = psB.tile([128, 260], F32, tag="ps_u")
                        for i, gg in enumerate(gl):
                            nc.tensor.matmul(
                                ps_u[:],
                                oh[:, gg * 128 : (gg + 1) * 128],
                                g_t[:, gg * RW : gg * RW + 260],
                                start=(i == 0),
                                stop=(i == len(gl) - 1),
                            )

                        r4 = sbB.tile([128, HEADS], F32, tag="r4")
                        nc.vector.tensor_scalar_add(r4[:], ps_u[:, 256:260], 1e-30)
                        nc.vector.reciprocal(r4[:], r4[:])
                        hp = sbB.tile([128, HEADS * HID], BF16, tag="hp")
                        nc.vector.tensor_tensor(
                            out=hp[:].rearrange("p (h o) -> p h o", h=HEADS),
                            in0=ps_u[:, 0 : HEADS * HID].rearrange(
                                "p (h o) -> p h o", h=HEADS
                            ),
                            in1=r4[:].unsqueeze(2).to_broadcast([128, HEADS, HID]),
                            op=ALU.mult,
                        )
                        # elu(x) = relu(x) + (exp(min(x,0)) - 1)
                        t0 = sbB.tile([128, HEADS * HID], BF16, tag="elu_t0")
                        nc.vector.tensor_scalar_min(t0[:], hp[:], 0.0)
                        t0e = sbB.tile([128, HEADS * HID], BF16, tag="elu_t0e")
                        nc.scalar.activation(t0e[:], t0[:], AF.Exp)
                        t1 = sbB.tile([128, HEADS * HID], BF16, tag="elu_t1")
                        nc.scalar.activation(t1[:], hp[:], AF.Relu)
                        he = sbB.tile([128, HEADS * HID], BF16, tag="he")
                        nc.vector.scalar_tensor_tensor(
                            he[:], t0e[:], -1.0, t1[:], ALU.add, ALU.add
                        )
                        nc.sync.dma_start(hcat[128 * w : 128 * w + wn, :], he[:wn, :])

        # ======== phase C: build own T2M / T2S1 shards ========
        with (
            tc.tile_pool(name="psC", bufs=2, space="PSUM") as psC,
            tc.tile_pool(name="sbC", bufs=3) as sbC,
            tc.tile_pool(name="cc", bufs=1) as ccpool,
        ):
            # W2ext chunks [128, 66] bf16 x2 : [W_out | c2o | c1o]
            w2e = []
            for c in range(2):
                w2c = ccpool.tile([128, OUT + 2], BF16, tag=f"w2e{c}")
                wo_t = sbC.tile([128, OUT], F32, tag="wo_t")
                nc.sync.dma_start(wo_t[:], wo_ext[128 * c : 128 * (c + 1), :])
                nc.scalar.copy(w2c[:, 0:OUT], wo_t[:])
                ps_w = psC.tile([OUT, 128], F32, tag="psw2")
                nc.tensor.transpose(ps_w[:], wo_t[:], identf[:])
                woT = sbC.tile([OUT, 128], F32, tag="woT")
                nc.vector.tensor_copy(woT[:], ps_w[:])
                aoc = sbC.tile([OUT, 2], F32, tag="aoc")
                nc.sync.dma_start(
                    aoc[:], ao_ext[:].unsqueeze(0).rearrange("1 (t o) -> o t", t=2)
                )
                ps_c2 = psC.tile([128, 2], F32, tag="psc2")
                nc.tensor.matmul(ps_c2[:], woT[:], aoc[:], start=True, stop=True)
                nc.vector.tensor_copy(w2c[:, OUT : OUT + 1], ps_c2[:, 1:2])
                nc.vector.tensor_copy(w2c[:, OUT + 1 : OUT + 2], ps_c2[:, 0:1])
                w2e.append(w2c)

            for _rc in range(reps.get("C", 1)):
                for t0, kk in _chunks(NW):
                    n0 = 128 * t0
                    rows = min(128 * kk, npc - n0)
                    full = min(kk, (npc - n0) // 128)
                    ht4 = sbC.tile([128, TB * HEADS * HID], BF16, tag="ht4")
                    if full:
                        nc.sync.dma_start(
                            ht4[:, : 256 * full].rearrange("p (k c) -> p k c", c=256),
                            hcat[n0 : n0 + 128 * full, :].rearrange(
                                "(k p) c -> p k c", p=128
                            ),
                        )
                    if full < kk:
                        rem = npc - n0 - 128 * full
                        nc.sync.dma_start(
                            ht4[:rem, 256 * full : 256 * full + 256],
                            hcat[n0 + 128 * full : npc, :],
                        )
                    ot = sbC.tile([128, TB * 66], BF16, tag="otC")
                    os4 = sbC.tile([128, TB], F32, tag="osC")
                    for k in range(kk):
                        ps_o = psC.tile([128, OUT + 2], F32, tag="psC_o")
                        for c in range(2):
                            ps_t = psC.tile([128, 128], BF16, tag="psC_t")
                            nc.tensor.transpose(
                                ps_t[:],
                                ht4[:, 256 * k + 128 * c : 256 * k + 128 * (c + 1)],
                                identb[:],
                            )
                            hT = sbC.tile([128, 128], BF16, tag="hT")
                            nc.scalar.copy(hT[:], ps_t[:])
                            nc.tensor.matmul(
                                ps_o[:], hT[:], w2e[c][:], start=(c == 0), stop=(c == 1)
                            )
                        nc.scalar.copy(
                            ot[:, 66 * k : 66 * k + OUT], ps_o[:, 0:OUT]
                        )
                        nc.vector.tensor_copy(
                            ot[:, 66 * k + 64 : 66 * k + 66].bitcast(F32),
                            ps_o[:, OUT : OUT + 1],
                        )
                        nc.vector.tensor_copy(
                            os4[:, k : k + 1], ps_o[:, OUT + 1 : OUT + 2]
                        )
                    if full:
                        nc.sync.dma_start(
                            t2msh[n0 : n0 + 128 * full, 0:66].rearrange(
                                "(k p) c -> p k c", p=128
                            ),
                            ot[:, : 66 * full].rearrange("p (k c) -> p k c", c=66),
                        )
                        nc.sync.dma_start(
                            t2s1[n0 : n0 + 128 * full, 0:1].rearrange(
                                "(k p) c -> p k c", p=128
                            ),
                            os4[:, :full].rearrange("p (k c) -> p k c", c=1),
                        )
                    if full < kk:
                        rem = npc - n0 - 128 * full
                        nc.sync.dma_start(
                            t2msh[n0 + 128 * full : npc, 0:66],
                            ot[:rem, 66 * full : 66 * full + 66],
                        )
                        nc.sync.dma_start(
                            t2s1[n0 + 128 * full : npc, 0:1],
                            os4[:rem, full : full + 1],
                        )

        # ======== phase D: allgather T2M ========
        if cfg.cores > 1 and not mock_d:
            nc.gpsimd.collective_compute(
                "AllGather",
                ALU.bypass,
                replica_groups=[list(range(cfg.cores))],
                ins=[t2msh[:]],
                outs=[t2m[1 : n + 1, :]],
            )
        else:
            nc.sync.dma_start(t2m[1 : npc + 1, :], t2msh[:])

        # ======== phase E: layer-2 edge processing ========
        t2mh = t2m[HALF + 1 :, :] if ANYB else None
        with (
            tc.tile_pool(name="psE", bufs=4, space="PSUM") as psE,
            tc.tile_pool(name="sbE", bufs=2) as sbE,
            tc.tile_pool(name="sbEi", bufs=2) as sbEi,
        ):
            for _re in range(reps.get("E", 1)):
                for bi, bt in enumerate(cfg.batches):
                    w0, kw, BG, SA = bt["w0"], bt["kw"], bt["bg"], bt["sa"]
                    i16d = sbEi.tile([128, MBG * 8], I16, tag="i16d")
                    nc.sync.dma_start(i16d[:, : BG * 8], idx_d16[bi, :, : BG * 8])
                    i16s = sbEi.tile([128, MBG * 8], I16, tag="i16s")
                    nc.sync.dma_start(i16s[:, : BG * 8], idx_s16[bi, :, : BG * 8])
                    if HOSTOH:
                        oh = sbEi.tile([128, MBG * 128], BF16, tag="oh")
                        nc.sync.dma_start(
                            oh[:, : BG * 128], oh16[bi, :, : BG * 128]
                        )
                    else:
                        srcl = sbEi.tile([128, MBG], I32, tag="srcl")
                        nc.sync.dma_start(srcl[:, :BG], idx_srcl[bi, :, :BG])
                        srclf = sbEi.tile([128, MBG], BF16, tag="srclf")
                        nc.vector.tensor_copy(srclf[:, :BG], srcl[:, :BG])
                        oh = sbE.tile([128, MBG * 128], BF16, tag="oh")
                        nc.vector.tensor_tensor(
                            out=oh[:, : BG * 128].rearrange(
                                "p (g j) -> p g j", j=128
                            ),
                            in0=srclf[:, :BG]
                            .unsqueeze(2)
                            .to_broadcast([128, BG, 128]),
                            in1=iota_t[:, : BG * 128].rearrange(
                                "p (g j) -> p g j", j=128
                            ),
                            op=ALU.is_equal,
                        )

                    g2 = sbE.tile([128, MBG * R2W], BF16, tag="g_t2")
                    for c0, q in _gcalls(0, SA):
                        nc.gpsimd.dma_gather(
                            g2[:, c0 * R2W : (c0 + q) * R2W].rearrange(
                                "p (k e) -> p k e", e=R2W
                            ),
                            t2m[:],
                            i16d[:, c0 * 8 : (c0 + q) * 8],
                            q * 128,
                            q * 128,
                            R2W,
                            queue_num=_q(),
                        )
                    for c0, q in _gcalls(SA, BG):
                        nc.gpsimd.dma_gather(
                            g2[:, c0 * R2W : (c0 + q) * R2W].rearrange(
                                "p (k e) -> p k e", e=R2W
                            ),
                            t2mh,
                            i16d[:, c0 * 8 : (c0 + q) * 8],
                            q * 128,
                            q * 128,
                            R2W,
                            queue_num=_q(),
                        )
                    s1e = sbE.tile([128, MBG * 64], F32, tag="s1e2")
                    for c0, q in _gcalls(0, BG):
                        nc.gpsimd.dma_gather(
                            s1e[:, c0 * 64 : (c0 + q) * 64].rearrange(
                                "p (k e) -> p k e", e=64
                            ),
                            t2s1[:],
                            i16s[:, c0 * 8 : (c0 + q) * 8],
                            q * 128,
                            q * 128,
                            64,
                            queue_num=_q(),
                        )

                    g23 = g2[:, : BG * R2W].rearrange("p (g c) -> p g c", c=R2W)
                    g23f = (
                        g2[:, : BG * R2W]
                        .bitcast(F32)
                        .rearrange("p (g c) -> p g c", c=64)
                    )
                    s13 = s1e[:, : BG * 64].rearrange("p (g c) -> p g c", c=64)
                    e_t = sbE.tile([128, MBG], F32, tag="e_t2")
                    nc.vector.tensor_add(
                        e_t[:, :BG].unsqueeze(2), s13[:, :, 0:1], g23f[:, :, 32:33]
                    )
                    lr_t = sbE.tile([128, MBG], F32, tag="lr_t2")
                    nc.vector.tensor_scalar_mul(lr_t[:, :BG], e_t[:, :BG], ALPHA)
                    nc.vector.tensor_tensor(
                        lr_t[:, :BG], e_t[:, :BG], lr_t[:, :BG], op=ALU.max
                    )
                    ex_b = sbE.tile([128, MBG], BF16, tag="ex_b2")
                    nc.scalar.activation(ex_b[:, :BG], lr_t[:, :BG], AF.Exp)
                    nc.vector.tensor_tensor(
                        out=g23[:, :, 0:OUT],
                        in0=g23[:, :, 0:OUT],
                        in1=ex_b[:, :BG].unsqueeze(2).to_broadcast([128, BG, OUT]),
                        op=ALU.mult,
                    )
                    nc.vector.tensor_copy(
                        g23[:, :, OUT : OUT + 1], ex_b[:, :BG].unsqueeze(2)
                    )

                    for kwi in range(kw):
                        w = w0 + kwi
                        wn = min(128, npc - 128 * w)
                        gl = list(
                            range(
                                bt["aoff"][kwi], bt["aoff"][kwi] + bt["kas"][kwi]
                            )
                        ) + list(
                            range(
                                bt["boff"][kwi], bt["boff"][kwi] + bt["kbs"][kwi]
                            )
                        )
                        ps_u = psE.tile([128, OUT + 1], F32, tag="ps_u2")
                        for i, gg in enumerate(gl):
                            nc.tensor.matmul(
                                ps_u[:],
                                oh[:, gg * 128 : (gg + 1) * 128],
                                g2[:, gg * R2W : gg * R2W + OUT + 1],
                                start=(i == 0),
                                stop=(i == len(gl) - 1),
                            )

                        r1 = sbE.tile([128, 1], F32, tag="r12")
                        nc.vector.tensor_scalar_add(
                            r1[:], ps_u[:, OUT : OUT + 1], 1e-30
                        )
                        nc.vector.reciprocal(r1[:], r1[:])
                        op_t = sbE.tile([128, OUT], F32, tag="op_t")
                        nc.vector.tensor_tensor(
                            out=op_t[:],
                            in0=ps_u[:, 0:OUT],
                            in1=r1[:].to_broadcast([128, OUT]),
                            op=ALU.mult,
                        )
                        t0 = sbE.tile([128, OUT], F32, tag="elu2_t0")
                        nc.vector.tensor_scalar_min(t0[:], op_t[:], 0.0)
                        t0e = sbE.tile([128, OUT], F32, tag="elu2_t0e")
                        nc.scalar.activation(t0e[:], t0[:], AF.Exp)
                        t1 = sbE.tile([128, OUT], F32, tag="elu2_t1")
                        nc.scalar.activation(t1[:], op_t[:], AF.Relu)
                        oe = sbE.tile([128, OUT], F32, tag="oe")
                        nc.vector.scalar_tensor_tensor(
                            oe[:], t0e[:], -1.0, t1[:], ALU.add, ALU.add
                        )
                        nc.sync.dma_start(
                            out_ext[128 * w : 128 * w + wn, :], oe[:wn, :]
                        )

        if dbg:
            nc.sync.dma_start(dbg_tw[:], tw[:])
            nc.sync.dma_start(dbg_ts1[:], ts1[:])
            nc.sync.dma_start(dbg_hcat[:], hcat[:])
            nc.sync.dma_start(dbg_t2m[:], t2m[:])
            nc.sync.dma_start(dbg_t2s1[:], t2s1[:])

    nc.compile()
    return nc


# ---------------------------------------------------------------------------
# Host-side preparation and execution
# ---------------------------------------------------------------------------


def _pack16_slots(slot_vals, nw, g):
    """slot_vals [NW, G*128] in slot order j -> [NW, 128, G*8] int16 layout:
    idx j at [16*r + j%16, j//16], replicated for r in 0..7."""
    w = slot_vals.reshape(nw, g * 8, 16)  # [NW, j//16, j%16]
    w = np.swapaxes(w, 1, 2)  # [NW, 16, G*8]
    return np.ascontiguousarray(np.tile(w, (1, 8, 1)).astype(np.int16))


def section_sizes(n, cores, edges):
    """Per-window-index max A/B-section group counts over cores."""
    npc = n // cores
    nw = (npc + 127) // 128
    src = np.asarray(edges[0], dtype=np.int64)
    dst = np.asarray(edges[1], dtype=np.int64)
    order = np.argsort(src, kind="stable")
    ssrc, sdst = src[order], dst[order]
    kaw = np.zeros(nw, dtype=np.int64)
    kbw = np.zeros(nw, dtype=np.int64)
    for k in range(cores):
        for w in range(nw):
            lo = k * npc + 128 * w
            hi = min(lo + 128, (k + 1) * npc)
            s0, s1 = np.searchsorted(ssrc, [lo, hi])
            d = sdst[s0:s1]
            ca = int((d < HALF).sum())
            cb = int(len(d) - ca)
            kaw[w] = max(kaw[w], max(1, -(-ca // 128)))
            kbw[w] = max(kbw[w], -(-cb // 128))
    if n + 1 > HALF:
        kbw = np.maximum(kbw, 1)
    return kaw, kbw


def prepare_inputs(cfg: Cfg, x, edges, W_heads, a_heads, W_out, a_out):
    """Build per-core input maps. Pure layout/index manipulation.

    Gather-batch layout: windows are grouped WB at a time; within a batch the
    slot/group order is [A-sec w0 | A-sec w1 | ... | B-sec w0 | B-sec w1 ...]
    with per-window group counts cfg.kaw/kbw, so each section is one
    contiguous span of dma_gather calls over all kw windows."""
    n, cores, npc, NW = cfg.n, cfg.cores, cfg.npc, cfg.nw
    NB, MBG = cfg.nb, cfg.maxbg
    src = np.asarray(edges[0], dtype=np.int64)
    dst = np.asarray(edges[1], dtype=np.int64)
    order = np.argsort(src, kind="stable")
    ssrc = src[order]
    sdst = dst[order]

    xT = np.ascontiguousarray(np.asarray(x, np.float32).T.astype(BF_NP))

    common = dict(
        xT=xT,
        W_heads=np.asarray(W_heads, np.float32),
        a_heads=np.asarray(a_heads, np.float32),
        W_out=np.asarray(W_out, np.float32),
        a_out=np.asarray(a_out, np.float32),
    )

    ar128 = np.arange(128, dtype=np.int32)
    in_maps = []
    for k in range(cores):
        vd = np.zeros((NB, MBG * 128), dtype=np.int64)
        vs = np.zeros((NB, MBG * 128), dtype=np.int64)
        if HOSTOH:
            oh = np.zeros((NB, 128, MBG * 128), dtype=BF_NP)
        else:
            vl = np.zeros((NB, 128, MBG), dtype=np.int32)
        for bi, bt in enumerate(cfg.batches):
            w0, kw = bt["w0"], bt["kw"]
            dA, sA, wA = [], [], []
            dB, sB, wB = [], [], []
            for kwi in range(kw):
                w = w0 + kwi
                ka, kb = bt["kas"][kwi], bt["kbs"][kwi]
                lo = k * npc + 128 * w
                hi = min(lo + 128, (k + 1) * npc)
                s0, s1 = np.searchsorted(ssrc, [lo, hi])
                d, sr = sdst[s0:s1], ssrc[s0:s1]
                selA = d < HALF
                da, sa = d[selA], sr[selA]
                db, sb = d[~selA], sr[~selA]
                assert len(da) <= 128 * ka and len(db) <= 128 * kb
                rowA = np.zeros(128 * ka, dtype=np.int64)
                rowA[: len(da)] = da + 1
                dA.append(rowA)
                rowB = np.full(128 * kb, n + 1 - (HALF + 1), dtype=np.int64)
                rowB[: len(db)] = db + 1 - (HALF + 1)
                dB.append(rowB)
                sa_l = np.zeros(128 * ka, dtype=np.int64)
                sa_l[: len(sa)] = sa - k * npc
                sA.append(sa_l)
                sb_l = np.zeros(128 * kb, dtype=np.int64)
                sb_l[: len(sb)] = sb - k * npc
                sB.append(sb_l)
                wa = np.zeros(128 * ka, dtype=np.int32)
                wa[: len(sa)] = (sa - lo).astype(np.int32)
                wA.append(wa)
                wb_ = np.zeros(128 * kb, dtype=np.int32)
                wb_[: len(sb)] = (sb - lo).astype(np.int32)
                wB.append(wb_)
            vdb = np.concatenate(dA + dB)
            vsb = np.concatenate(sA + sB)
            vd[bi, : len(vdb)] = vdb
            vs[bi, : len(vsb)] = vsb
            wl = np.concatenate(wA + wB)  # [BG*128] slot-major (g, p)
            wlT = wl.reshape(-1, 128).T  # [128, BG]
            if HOSTOH:
                ohb = (wlT[:, :, None] == ar128[None, None, :]).astype(BF_NP)
                oh[bi, :, : ohb.shape[1] * 128] = ohb.reshape(128, -1)
            else:
                vl[bi, :, : wlT.shape[1]] = wlT
        in_maps.append(
            dict(
                common,
                xTown=np.ascontiguousarray(xT[:, k * npc : (k + 1) * npc]),
                idx_d16=_pack16_slots(vd, NB, MBG),
                idx_s16=_pack16_slots(vs, NB, MBG),
                **({"oh16": oh} if HOSTOH else {"idx_srcl": vl}),
            )
        )
    return in_maps


_NC_CACHE = {}


def get_nc(cfg: Cfg):
    if cfg.key not in _NC_CACHE:
        _NC_CACHE[cfg.key] = build_nc(cfg)
    return _NC_CACHE[cfg.key]


def make_cfg(n, cores, edges):
    kaw, kbw = section_sizes(n, cores, edges)
    if UNIKA:
        kaw = np.full_like(kaw, kaw.max())
        kbw = np.full_like(kbw, kbw.max())
    return Cfg(n, cores, kaw, kbw)


def run(inputs, trace=False, **spmd_kwargs):
    from concourse.bass_utils import run_bass_kernel_spmd

    x = np.asarray(inputs["x"], np.float32)
    edges = np.asarray(inputs["edges"])
    cfg = make_cfg(N, CORES, edges)
    nc = get_nc(cfg)
    in_maps = prepare_inputs(
        cfg,
        x,
        edges,
        inputs["W_heads"],
        inputs["a_heads"],
        inputs["W_out"],
        inputs["a_out"],
    )
    res = run_bass_kernel_spmd(
        nc, in_maps, core_ids=list(range(CORES)), trace=trace, **spmd_kwargs
    )
    out = np.concatenate([r["out"] for r in res.results], axis=0)
    return out, res


def kernel(**inputs):
    return run(inputs)[0]
